# revision 11
# baseline (speedup 1.0000x reference)
"""Causal shaped attention kernel for Trainium2 (8 NeuronCores).

y = beta * softmax(causal(q k^T / 8)) @ v + alpha * Id @ v - gamma * MC @ v
  with q,k = x @ w_attn.T split, v = x, Id = softmax(eye(T)), MC = causal row-mean.

Sharding: (batch, head-group) across 8 cores: core c -> b = c//2, heads
h0 = (c%2)*8 .. h0+8.  Each core computes y[b, :, h0*64 : h0*64+512].

v5: all matmul inputs bf16 (f32 PSUM accumulation), host pre-transposed
x/W; s-chunk-streamed projection, B2 colsum/cumsum batched across heads
and woven between projection chunks; phase-C strips software-pipelined at
block level (S-block of strip i, then AV-block of strip i-1) with the
S-pair PSUM ring double-buffered so the PE never waits on exp drains.

Id@v and MC@v closed forms (no T x T materialization):
  Id@v[i] = ((e-1) v[i] + colsum(v)) / (e+T-1)
  MC@v[i] = cumsum(v)[i] / (i+1)
"""

import sys

if "/opt/trn_rl_repo" not in sys.path:
    sys.path.insert(0, "/opt/trn_rl_repo")

import math

import numpy as np
import ml_dtypes

import concourse.bass as bass
import concourse.mybir as mybir
import concourse.tile as tile
from concourse import bacc
from concourse.bass_utils import run_bass_kernel_spmd

F32 = mybir.dt.float32
BF16 = mybir.dt.bfloat16
AF = mybir.ActivationFunctionType
OP = mybir.AluOpType

N_CORES = 8
B, T, C = 4, 2048, 1024
NHC = 8          # heads per core
NT = T // 128    # 16 j/i tiles
NS = 4           # i-strips of 512
CONSTS_W = 276   # 128 tril + 16 negipg + k1 + k2 + kb + pad + 128 ident

_NC_CACHE = {}


def emit(nc, tc, xT_d, wt_d, vones_d, consts, yout):
    pools = []

    def pool(name, **kw):
        p = tc.alloc_tile_pool(name=name, **kw)
        pools.append(p)
        return p

    cpool = pool("cpool", bufs=1)
    ps = pool("ps", bufs=2, space="PSUM")

    cons = cpool.tile([128, CONSTS_W], F32, name="cons")
    nc.sync.dma_start(out=cons[:], in_=consts[:])
    tril = cons[:, 0:128]
    ident = cons[:, 148:276]
    negipg = cons[:, 128:144]      # [128, 16] : -gamma/(i+1)
    k1c = cons[:, 144:145]
    kbc = cons[:, 146:147]
    trilb = cpool.tile([128, 128], BF16, name="trilb")
    nc.gpsimd.tensor_copy(out=trilb[:], in_=tril)
    ones_row = trilb[0:1, 0:128]   # tril row 0 == all ones (K=1 lhsT)
    ones_col = trilb[:, 127:128]   # tril col 127 == all ones [128, 1]

    # ---------------- persistent SBUF ----------------
    big = pool("big", bufs=1)
    WT = big.tile([128, 2, 4, 8, 128], BF16, name="WT")
    vones = big.tile([128, NHC, NT, 65], BF16, name="vones")
    xc = big.tile([128, 8, 2048], BF16, name="xc")
    qkT = big.tile([128, 4, 2, 2048], BF16, name="qkT")
    static = big.tile([128, NT, 512], F32, name="static")
    runs = big.tile([1, 2, 512], F32, name="runs")
    runb = big.tile([1, NT, 512], BF16, name="runb")
    colb = big.tile([128, 512], F32, name="colb")

    wtv = wt_d[:].rearrange("p (a b c d) -> p a b c d", a=2, b=4, c=8)
    xTv = xT_d[:].rearrange("p (a t) -> p a t", a=8)
    nc.sync.dma_start(out=WT[:, :, 0, :, :], in_=wtv[:, :, 0, :, :])
    nc.sync.dma_start(out=xc[:, :, 0:512], in_=xTv[:, :, 0:512])
    nc.sync.dma_start(out=vones[:], in_=vones_d[:].rearrange(
        "p (a b c) -> p a b c", a=NHC, b=NT))
    nc.sync.dma_start(out=WT[:, :, 1:4, :, :], in_=wtv[:, :, 1:4, :, :])
    for s in range(1, NS):
        nc.sync.dma_start(out=xc[:, :, s * 512:(s + 1) * 512],
                          in_=xTv[:, :, s * 512:(s + 1) * 512])

    # warm the PE clock (HAM) during the input-DMA window: dummy matmuls on
    # already-resident constants so projections start at 2.4 GHz
    for _ in range(24):
        wm = ps.tile([128, 2, 512], F32, name="wm", tag="ps2", bufs=2)
        for u in range(2):
            nc.tensor.matmul(wm[:, u, 0:128], trilb[:], trilb[:],
                             start=True, stop=True)

    def proj_chunk(s):
        for p in range(4):
            pj2 = ps.tile([128, 2, 512], F32, name="pj2", tag="ps2", bufs=2)
            for qk in range(2):
                for ci in range(8):
                    nc.tensor.matmul(pj2[:, qk, :], WT[:, qk, p, ci, :],
                                     xc[:, ci, s * 512:(s + 1) * 512],
                                     start=(ci == 0), stop=(ci == 7))
            nc.vector.tensor_copy(out=qkT[:, p, 0:2, s * 512:(s + 1) * 512],
                                  in_=pj2[:])

    # B2a: per-tile colsums (batched over heads, strided rhs) + serial prefix
    def b2_colsums():
        nc.vector.memset(runs[0:1, 0, :], 0.0)
        for I in range(NT):
            nc.vector.tensor_copy(out=runb[0:1, I, :], in_=runs[0:1, I % 2, :])
            cpt = ps.tile([1, 512], F32, name="cpt", tag="cs", bufs=1)
            nc.tensor.matmul(cpt[0:1, :], ones_col, vones[:, :, I, 0:64],
                             start=True, stop=True)
            nc.vector.tensor_add(runs[0:1, (I + 1) % 2, :],
                                 runs[0:1, I % 2, :], cpt[0:1, :])
        # colb = k2 * total colsum, broadcast to all partitions
        nc.vector.tensor_scalar(out=runs[0:1, 1, :], in0=runs[0:1, NT % 2, :],
                                scalar1=cons[0:1, 145:146], scalar2=None,
                                op0=OP.mult)
        nc.gpsimd.partition_broadcast(colb[:], runs[0:1, 1, :])

    # B2b: cumsum tiles + static combine
    def b2_static():
        for i0 in range(0, NT, 2):
            cu2 = ps.tile([128, 2, 512], F32, name="cu2", tag="ps2", bufs=2)
            for u, I in ((0, i0), (1, i0 + 1)):
                nc.tensor.matmul(cu2[:, u, :], ones_row, runb[0:1, I, :],
                                 start=True, stop=False)
                nc.tensor.matmul(cu2[:, u, :], trilb[:], vones[:, :, I, 0:64],
                                 start=False, stop=True)
            for u, I in ((0, i0), (1, i0 + 1)):
                nc.vector.scalar_tensor_tensor(
                    out=static[:, I, :].rearrange("p (h d) -> p h d", h=NHC),
                    in0=vones[:, :, I, 0:64],
                    scalar=k1c, in1=colb[:].rearrange("p (h d) -> p h d", h=NHC),
                    op0=OP.mult, op1=OP.add)
                nc.vector.scalar_tensor_tensor(
                    out=static[:, I, :], in0=cu2[:, u, :],
                    scalar=negipg[:, I:I + 1],
                    in1=static[:, I, :], op0=OP.mult, op1=OP.add)

    # ---------------- phase C: attention, software-pipelined strips ---------
    ptp = pool("ptp", bufs=2)
    ysp = pool("ysp", bufs=2)

    def s_block(hh, g, pt):
        p, base = hh // 2, (hh % 2) * 64
        qT = qkT[base:base + 64, p, 0, :]
        kT = qkT[base:base + 64, p, 1, :]
        nj = 4 * g + 4
        meta = []
        for J in range(0, nj, 2):
            st2 = ps.tile([128, 2, 512], F32, name="st2", tag="ps2", bufs=2)
            diag = J >= 4 * g
            if not diag:
                for u in range(2):
                    nc.tensor.matmul(
                        st2[:, u, :], kT[:, (J + u) * 128:(J + u + 1) * 128],
                        qT[:, g * 512:(g + 1) * 512],
                        start=True, stop=True)
                nc.scalar.activation(out=pt[:, J:J + 2, :], in_=st2[:],
                                     func=AF.Exp, scale=0.125)
                meta.append((J, 0))
                meta.append((J + 1, 0))
            else:
                offs = (128 * (J - 4 * g), 128 * (J + 1 - 4 * g))
                for u in range(2):
                    io = offs[u]
                    nc.tensor.matmul(
                        st2[:, u, io:512], kT[:, (J + u) * 128:(J + u + 1) * 128],
                        qT[:, g * 512 + io:(g + 1) * 512],
                        start=True, stop=True)
                for u in range(2):
                    io = offs[u]
                    nc.scalar.activation(out=pt[:, J + u, io:512],
                                         in_=st2[:, u, io:512],
                                         func=AF.Exp, scale=0.125)
                    # diagonal tile: keep j <= i only
                    nc.gpsimd.tensor_mul(pt[:, J + u, io:io + 128],
                                         pt[:, J + u, io:io + 128], trilb[:])
                    meta.append((J + u, io))
        return meta

    def av_block(hh, g, pt, meta):
        nj = len(meta)
        yps = ps.tile([128, 512], F32, name="yps", tag="yps", bufs=2)
        for J, i_off in meta:
            nc.tensor.matmul(
                yps[0:65, i_off:512], vones[:, hh, J, :],
                pt[:, J, i_off:512],
                start=(J == 0), stop=(J == nj - 1), skip_group_check=True)
        # evacuate y^T [65, 512], transpose back to [i, 65]
        ysb = ysp.tile([65, 512], F32, name="ysb", tag="ysb")
        nc.vector.tensor_copy(out=ysb[:], in_=yps[0:65, :])
        tp = ps.tile([128, 260], F32, name="tp", tag="tp", bufs=1)
        for k in range(4):
            nc.tensor.transpose(tp[:, k * 65:(k + 1) * 65],
                                ysb[:, k * 128:(k + 1) * 128], ident[0:65, 0:65])
        rc4 = ysp.tile([128, 4], F32, name="rc4", tag="rc4")
        nc.vector.reciprocal(out=rc4[:], in_=tp[:, 64:260:65])
        nc.vector.tensor_scalar(out=rc4[:], in0=rc4[:], scalar1=kbc,
                                scalar2=None, op0=OP.mult)
        yo = ysp.tile([128, 4, 64], F32, name="yo", tag="yo")
        for k in range(4):
            nc.vector.scalar_tensor_tensor(
                out=yo[:, k, :], in0=tp[:, k * 65:k * 65 + 64],
                scalar=rc4[:, k:k + 1],
                in1=static[:, 4 * g + k, hh * 64:(hh + 1) * 64],
                op0=OP.mult, op1=OP.add)
        nc.sync.dma_start(
            out=yout[g * 512:(g + 1) * 512, hh * 64:(hh + 1) * 64]
            .rearrange("(k p) d -> p k d", p=128),
            in_=yo[:])

    def attn_pair(p):
        # heavy strips first; interleave the pair's two heads
        strips = [(2 * p + half, g) for g in (3, 2, 1, 0) for half in (0, 1)]
        pending = None
        for hh, g in strips:
            pt = ptp.tile([128, 16, 512], BF16, name="pt", tag="pt")
            meta = s_block(hh, g, pt)
            if pending is not None:
                av_block(*pending)
            pending = (hh, g, pt, meta)
        av_block(*pending)

    proj_chunk(0)
    proj_chunk(1)
    b2_colsums()
    proj_chunk(2)
    proj_chunk(3)
    b2_static()
    for p in range(4):
        attn_pair(p)

    for p in reversed(pools):
        p.release()


def build_nc():
    if "nc" in _NC_CACHE:
        return _NC_CACHE["nc"]
    nc = bacc.Bacc("TRN2", target_bir_lowering=False)
    xT_d = nc.declare_dram_parameter("xT", [128, 8 * 2048], BF16, isOutput=False)
    wt_d = nc.declare_dram_parameter("wt", [128, 2 * 4 * 8 * 128], BF16, isOutput=False)
    vones_d = nc.declare_dram_parameter("vones", [128, NHC * NT * 65], BF16, isOutput=False)
    consts = nc.declare_dram_parameter("consts", [128, CONSTS_W], F32, isOutput=False)
    yout = nc.declare_dram_parameter("yout", [T, 512], F32, isOutput=True)
    with tile.TileContext(nc) as tc:
        emit(nc, tc, xT_d, wt_d, vones_d, consts, yout)
    nc.compile()
    _NC_CACHE["nc"] = nc
    return nc


def make_consts(alpha, beta, gamma):
    D = math.e + T - 1
    k1 = alpha * (math.e - 1.0) / D
    k2 = alpha / D
    cons = np.zeros((128, CONSTS_W), dtype=np.float32)
    jj = np.arange(128)
    cons[:, 0:128] = (jj[:, None] <= jj[None, :]).astype(np.float32)  # tril mask
    for I in range(16):
        cons[:, 128 + I] = -gamma / (128.0 * I + jj + 1.0)
    cons[:, 144] = k1
    cons[:, 145] = k2
    cons[:, 146] = beta
    cons[:, 148:276] = np.eye(128, dtype=np.float32)
    return cons


def kernel(x, w_attn, alpha, beta, gamma, _trace=False):
    x = np.asarray(x, dtype=np.float32)
    w_attn = np.asarray(w_attn, dtype=np.float32)
    alpha = float(np.asarray(alpha))
    beta = float(np.asarray(beta))
    gamma = float(np.asarray(gamma))

    nc = build_nc()
    cons = make_consts(alpha, beta, gamma)
    bf = ml_dtypes.bfloat16
    in_maps = []
    for c in range(N_CORES):
        b, h0 = c // 2, (c % 2) * 8
        c0 = h0 * 64
        # rotate columns of x and w so this core's v-block sits at columns 0:512
        # (the projection q,k = x @ w.T is invariant to a consistent column roll)
        xb_r = np.roll(x[b], -c0, axis=1)
        xT_h = np.ascontiguousarray(
            xb_r.T.reshape(8, 128, 2048).transpose(1, 0, 2)).reshape(128, -1)
        wqk = np.concatenate(
            [w_attn[c0: c0 + 512], w_attn[C + c0: C + c0 + 512]], axis=0)
        wqk_r = np.roll(wqk, -c0, axis=1)
        # [cp, qk, p, ci, m] <- wqk_r[qk*512 + p*128 + m, ci*128 + cp]
        wt_h = np.ascontiguousarray(
            wqk_r.T.reshape(8, 128, 2, 4, 128).transpose(1, 2, 3, 0, 4)).reshape(128, -1)
        vo = np.ones((128, NHC, NT, 65), dtype=bf)
        vo[:, :, :, 0:64] = x[b][:, c0:c0 + 512].reshape(16, 128, 8, 64).transpose(
            1, 2, 0, 3).astype(bf)
        in_maps.append({"xT": xT_h.astype(bf), "wt": wt_h.astype(bf),
                        "vones": vo.reshape(128, -1), "consts": cons})
    res = run_bass_kernel_spmd(nc, in_maps, list(range(N_CORES)), trace=_trace)
    y = np.empty((B, T, C), dtype=np.float32)
    for c in range(N_CORES):
        b, h0 = c // 2, (c % 2) * 8
        y[b, :, h0 * 64: h0 * 64 + 512] = res.results[c]["yout"]
    if _trace:
        kernel.last_exec_time_ns = res.exec_time_ns
    return y


# revision 12
# speedup vs baseline: 1.1230x; 1.1230x over previous
"""Causal shaped attention kernel for Trainium2 (8 NeuronCores).

y = beta * softmax(causal(q k^T / 8)) @ v + alpha * Id @ v - gamma * MC @ v
  with q,k = x @ w_attn.T split, v = x, Id = softmax(eye(T)), MC = causal row-mean.

Sharding: (batch, head-group) across 8 cores: core c -> b = c//2, heads
h0 = (c%2)*8 .. h0+8.  Each core computes y[b, :, h0*64 : h0*64+512].

v5: all matmul inputs bf16 (f32 PSUM accumulation), host pre-transposed
x/W; s-chunk-streamed projection, B2 colsum/cumsum batched across heads
and woven between projection chunks; phase-C strips software-pipelined at
block level (S-block of strip i, then AV-block of strip i-1) with the
S-pair PSUM ring double-buffered so the PE never waits on exp drains.

Id@v and MC@v closed forms (no T x T materialization):
  Id@v[i] = ((e-1) v[i] + colsum(v)) / (e+T-1)
  MC@v[i] = cumsum(v)[i] / (i+1)
"""

import sys

if "/opt/trn_rl_repo" not in sys.path:
    sys.path.insert(0, "/opt/trn_rl_repo")

import math

import numpy as np
import ml_dtypes

import concourse.bass as bass
import concourse.mybir as mybir
import concourse.tile as tile
from concourse import bacc
from concourse.bass_utils import run_bass_kernel_spmd

F32 = mybir.dt.float32
BF16 = mybir.dt.bfloat16
AF = mybir.ActivationFunctionType
OP = mybir.AluOpType

N_CORES = 8
B, T, C = 4, 2048, 1024
NHC = 8          # heads per core
NT = T // 128    # 16 j/i tiles
NS = 4           # i-strips of 512
CONSTS_W = 276   # 128 tril + 16 negipg + k1 + k2 + kb + pad + 128 ident

_NC_CACHE = {}


def emit(nc, tc, xT_d, wt_d, vones_d, consts, yout):
    pools = []

    def pool(name, **kw):
        p = tc.alloc_tile_pool(name=name, **kw)
        pools.append(p)
        return p

    cpool = pool("cpool", bufs=1)
    ps = pool("ps", bufs=2, space="PSUM")

    cons = cpool.tile([128, CONSTS_W], F32, name="cons")
    nc.sync.dma_start(out=cons[:], in_=consts[:])
    tril = cons[:, 0:128]
    ident = cons[:, 148:276]
    negipg = cons[:, 128:144]      # [128, 16] : -gamma/(i+1)
    k1c = cons[:, 144:145]
    kbc = cons[:, 146:147]
    trilb = cpool.tile([128, 128], BF16, name="trilb")
    nc.gpsimd.tensor_copy(out=trilb[:], in_=tril)
    ones_row = trilb[0:1, 0:128]   # tril row 0 == all ones (K=1 lhsT)
    ones_col = trilb[:, 127:128]   # tril col 127 == all ones [128, 1]

    # ---------------- persistent SBUF ----------------
    big = pool("big", bufs=1)
    WT = big.tile([128, 2, 4, 8, 128], BF16, name="WT")
    vones = big.tile([128, NHC, NT, 65], BF16, name="vones")
    xc = big.tile([128, 8, 2048], BF16, name="xc")
    qkT = big.tile([128, 4, 2, 2048], BF16, name="qkT")
    static = big.tile([128, NT, 512], F32, name="static")
    runs = big.tile([1, 2, 512], F32, name="runs")
    runb = big.tile([1, NT, 512], BF16, name="runb")
    colb = big.tile([128, 512], F32, name="colb")

    wtv = wt_d[:].rearrange("p (a b c d) -> p a b c d", a=2, b=4, c=8)
    xTv = xT_d[:].rearrange("p (a t) -> p a t", a=8)
    nc.sync.dma_start(out=WT[:, :, 0, :, :], in_=wtv[:, :, 0, :, :])
    nc.sync.dma_start(out=xc[:, :, 0:512], in_=xTv[:, :, 0:512])
    nc.sync.dma_start(out=vones[:], in_=vones_d[:].rearrange(
        "p (a b c) -> p a b c", a=NHC, b=NT))
    nc.sync.dma_start(out=WT[:, :, 1:4, :, :], in_=wtv[:, :, 1:4, :, :])
    for s in range(1, NS):
        nc.sync.dma_start(out=xc[:, :, s * 512:(s + 1) * 512],
                          in_=xTv[:, :, s * 512:(s + 1) * 512])

    # warm the PE clock (HAM) during the input-DMA window: dummy matmuls on
    # already-resident constants so projections start at 2.4 GHz
    for _ in range(24):
        wm = ps.tile([128, 2, 512], F32, name="wm", tag="ps2", bufs=1)
        for u in range(2):
            nc.tensor.matmul(wm[:, u, 0:128], trilb[:], trilb[:],
                             start=True, stop=True)

    def proj_chunk(s):
        for p in range(4):
            for qk in range(2):
                pj = ps.tile([128, 512], F32, name="pj", tag="ps")
                for ci in range(8):
                    nc.tensor.matmul(pj[:], WT[:, qk, p, ci, :],
                                     xc[:, ci, s * 512:(s + 1) * 512],
                                     start=(ci == 0), stop=(ci == 7))
                nc.vector.tensor_copy(out=qkT[:, p, qk, s * 512:(s + 1) * 512],
                                      in_=pj[:])

    # B2a: per-tile colsums (batched over heads, strided rhs) + serial prefix
    def b2_colsums():
        nc.vector.memset(runs[0:1, 0, :], 0.0)
        for I in range(NT):
            nc.vector.tensor_copy(out=runb[0:1, I, :], in_=runs[0:1, I % 2, :])
            cpt = ps.tile([1, 512], F32, name="cpt", tag="cs", bufs=1)
            nc.tensor.matmul(cpt[0:1, :], ones_col, vones[:, :, I, 0:64],
                             start=True, stop=True)
            nc.vector.tensor_add(runs[0:1, (I + 1) % 2, :],
                                 runs[0:1, I % 2, :], cpt[0:1, :])
        # colb = k2 * total colsum, broadcast to all partitions
        nc.vector.tensor_scalar(out=runs[0:1, 1, :], in0=runs[0:1, NT % 2, :],
                                scalar1=cons[0:1, 145:146], scalar2=None,
                                op0=OP.mult)
        nc.gpsimd.partition_broadcast(colb[:], runs[0:1, 1, :])

    # B2b: cumsum tiles + static combine
    def b2_static():
        for I in range(NT):
            cu = ps.tile([128, 512], F32, name="cu", tag="ps")
            nc.tensor.matmul(cu[:], ones_row, runb[0:1, I, :],
                             start=True, stop=False)
            nc.tensor.matmul(cu[:], trilb[:], vones[:, :, I, 0:64],
                             start=False, stop=True)
            nc.vector.scalar_tensor_tensor(
                out=static[:, I, :].rearrange("p (h d) -> p h d", h=NHC),
                in0=vones[:, :, I, 0:64],
                scalar=k1c, in1=colb[:].rearrange("p (h d) -> p h d", h=NHC),
                op0=OP.mult, op1=OP.add)
            nc.vector.scalar_tensor_tensor(
                out=static[:, I, :], in0=cu[:], scalar=negipg[:, I:I + 1],
                in1=static[:, I, :], op0=OP.mult, op1=OP.add)

    # ---------------- phase C: attention, software-pipelined strips ---------
    ptp = pool("ptp", bufs=2)
    ysp = pool("ysp", bufs=2)

    def s_block(hh, g, pt):
        p, base = hh // 2, (hh % 2) * 64
        qT = qkT[base:base + 64, p, 0, :]
        kT = qkT[base:base + 64, p, 1, :]
        nj = 4 * g + 4
        meta = []
        J = 0
        while J < nj:
            if J + 1 < 4 * g and J % 2 == 0:
                # two full-width j-tiles: one 2-bank psum, one exp
                st2 = ps.tile([128, 2, 512], F32, name="st2", tag="ps2", bufs=1)
                for u in range(2):
                    nc.tensor.matmul(
                        st2[:, u, :], kT[:, (J + u) * 128:(J + u + 1) * 128],
                        qT[:, g * 512:(g + 1) * 512],
                        start=True, stop=True)
                nc.scalar.activation(out=pt[:, J:J + 2, :], in_=st2[:],
                                     func=AF.Exp, scale=0.125)
                meta.append((J, 0))
                meta.append((J + 1, 0))
                J += 2
                continue
            i_off = max(0, 128 * J - 512 * g)
            st = ps.tile([128, 512], F32, name="st", tag="ps")
            nc.tensor.matmul(
                st[:, i_off:512], kT[:, J * 128:(J + 1) * 128],
                qT[:, g * 512 + i_off:(g + 1) * 512],
                start=True, stop=True)
            nc.scalar.activation(out=pt[:, J, i_off:512], in_=st[:, i_off:512],
                                 func=AF.Exp, scale=0.125)
            if J >= 4 * g:
                # diagonal tile: keep j <= i only
                nc.gpsimd.tensor_mul(pt[:, J, i_off:i_off + 128],
                                     pt[:, J, i_off:i_off + 128], trilb[:])
            meta.append((J, i_off))
            J += 1
        return meta

    def av_block(hh, g, pt, meta):
        nj = len(meta)
        yps = ps.tile([128, 512], F32, name="yps", tag="yps", bufs=2)
        for J, i_off in meta:
            nc.tensor.matmul(
                yps[0:65, i_off:512], vones[:, hh, J, :],
                pt[:, J, i_off:512],
                start=(J == 0), stop=(J == nj - 1), skip_group_check=True)
        # evacuate y^T [65, 512], transpose back to [i, 65]
        ysb = ysp.tile([65, 512], F32, name="ysb", tag="ysb")
        nc.vector.tensor_copy(out=ysb[:], in_=yps[0:65, :])
        tp = ps.tile([128, 260], F32, name="tp", tag="tp", bufs=1)
        for k in range(4):
            nc.tensor.transpose(tp[:, k * 65:(k + 1) * 65],
                                ysb[:, k * 128:(k + 1) * 128], ident[0:65, 0:65])
        rc4 = ysp.tile([128, 4], F32, name="rc4", tag="rc4")
        nc.vector.reciprocal(out=rc4[:], in_=tp[:, 64:260:65])
        nc.vector.tensor_scalar(out=rc4[:], in0=rc4[:], scalar1=kbc,
                                scalar2=None, op0=OP.mult)
        yo = ysp.tile([128, 4, 64], F32, name="yo", tag="yo")
        for k in range(4):
            nc.vector.scalar_tensor_tensor(
                out=yo[:, k, :], in0=tp[:, k * 65:k * 65 + 64],
                scalar=rc4[:, k:k + 1],
                in1=static[:, 4 * g + k, hh * 64:(hh + 1) * 64],
                op0=OP.mult, op1=OP.add)
        nc.sync.dma_start(
            out=yout[g * 512:(g + 1) * 512, hh * 64:(hh + 1) * 64]
            .rearrange("(k p) d -> p k d", p=128),
            in_=yo[:])

    def attn_pair(p):
        # heavy strips first; interleave the pair's two heads
        strips = [(2 * p + half, g) for g in (3, 2, 1, 0) for half in (0, 1)]
        pending = None
        for hh, g in strips:
            pt = ptp.tile([128, 16, 512], BF16, name="pt", tag="pt")
            meta = s_block(hh, g, pt)
            if pending is not None:
                av_block(*pending)
            pending = (hh, g, pt, meta)
        av_block(*pending)

    proj_chunk(0)
    proj_chunk(1)
    b2_colsums()
    proj_chunk(2)
    proj_chunk(3)
    b2_static()
    for p in range(4):
        attn_pair(p)

    for p in reversed(pools):
        p.release()


def build_nc():
    if "nc" in _NC_CACHE:
        return _NC_CACHE["nc"]
    nc = bacc.Bacc("TRN2", target_bir_lowering=False)
    xT_d = nc.declare_dram_parameter("xT", [128, 8 * 2048], BF16, isOutput=False)
    wt_d = nc.declare_dram_parameter("wt", [128, 2 * 4 * 8 * 128], BF16, isOutput=False)
    vones_d = nc.declare_dram_parameter("vones", [128, NHC * NT * 65], BF16, isOutput=False)
    consts = nc.declare_dram_parameter("consts", [128, CONSTS_W], F32, isOutput=False)
    yout = nc.declare_dram_parameter("yout", [T, 512], F32, isOutput=True)
    with tile.TileContext(nc) as tc:
        emit(nc, tc, xT_d, wt_d, vones_d, consts, yout)
    nc.compile()
    _NC_CACHE["nc"] = nc
    return nc


def make_consts(alpha, beta, gamma):
    D = math.e + T - 1
    k1 = alpha * (math.e - 1.0) / D
    k2 = alpha / D
    cons = np.zeros((128, CONSTS_W), dtype=np.float32)
    jj = np.arange(128)
    cons[:, 0:128] = (jj[:, None] <= jj[None, :]).astype(np.float32)  # tril mask
    for I in range(16):
        cons[:, 128 + I] = -gamma / (128.0 * I + jj + 1.0)
    cons[:, 144] = k1
    cons[:, 145] = k2
    cons[:, 146] = beta
    cons[:, 148:276] = np.eye(128, dtype=np.float32)
    return cons


def kernel(x, w_attn, alpha, beta, gamma, _trace=False):
    x = np.asarray(x, dtype=np.float32)
    w_attn = np.asarray(w_attn, dtype=np.float32)
    alpha = float(np.asarray(alpha))
    beta = float(np.asarray(beta))
    gamma = float(np.asarray(gamma))

    nc = build_nc()
    cons = make_consts(alpha, beta, gamma)
    bf = ml_dtypes.bfloat16
    in_maps = []
    for c in range(N_CORES):
        b, h0 = c // 2, (c % 2) * 8
        c0 = h0 * 64
        # rotate columns of x and w so this core's v-block sits at columns 0:512
        # (the projection q,k = x @ w.T is invariant to a consistent column roll)
        xb_r = np.roll(x[b], -c0, axis=1)
        xT_h = np.ascontiguousarray(
            xb_r.T.reshape(8, 128, 2048).transpose(1, 0, 2)).reshape(128, -1)
        wqk = np.concatenate(
            [w_attn[c0: c0 + 512], w_attn[C + c0: C + c0 + 512]], axis=0)
        wqk_r = np.roll(wqk, -c0, axis=1)
        # [cp, qk, p, ci, m] <- wqk_r[qk*512 + p*128 + m, ci*128 + cp]
        wt_h = np.ascontiguousarray(
            wqk_r.T.reshape(8, 128, 2, 4, 128).transpose(1, 2, 3, 0, 4)).reshape(128, -1)
        vo = np.ones((128, NHC, NT, 65), dtype=bf)
        vo[:, :, :, 0:64] = x[b][:, c0:c0 + 512].reshape(16, 128, 8, 64).transpose(
            1, 2, 0, 3).astype(bf)
        in_maps.append({"xT": xT_h.astype(bf), "wt": wt_h.astype(bf),
                        "vones": vo.reshape(128, -1), "consts": cons})
    res = run_bass_kernel_spmd(nc, in_maps, list(range(N_CORES)), trace=_trace)
    y = np.empty((B, T, C), dtype=np.float32)
    for c in range(N_CORES):
        b, h0 = c // 2, (c % 2) * 8
        y[b, :, h0 * 64: h0 * 64 + 512] = res.results[c]["yout"]
    if _trace:
        kernel.last_exec_time_ns = res.exec_time_ns
    return y
